# revision 1
# baseline (speedup 1.0000x reference)
"""APPNP (MLP + K-step personalized-pagerank propagation) on 8 TRN2 NeuronCores.

Strategy
--------
Host (numpy, outside HW-timed region):
  - compute degrees / symmetric norms from edge_index
  - shard nodes across 8 cores (12500 each, padded to 12544 = 98 blocks of 128)
  - shard edges by dst core; per core group edges by (dst block of 128, src
    table-chunk of 25088 rows); pad each (block, chunk) run to a fixed T_q
    (multiple of 128) so the device graph is static and SPMD-uniform
  - build int16 gather indices (16-partition-wrapped, replicated x8 for the
    gpsimd dma_gather ucode), per-slot dst-local values for the on-device
    selection-matrix build, and per-node scale vectors

Device (one Bass graph, SPMD on 8 cores):
  - MLP in bf16 on TensorE: h0^T tiles = W2^T @ relu(W1^T @ X^T + b1) + b2
  - per 128-node block: transpose h0^T -> node-major, write
      hp   = (alpha/(1-alpha)) * h0 / norm_dst     (DRAM, reused all iters)
      HS_0 = norm_src * h0                         (shard of gather table)
  - AllGather shard -> full 100352-row gather table (ping/pong in DRAM)
  - K=10 propagation iterations, per block b:
      gather 4x T_q rows (512B each) from the table via gpsimd dma_gather
      build S[slot, dst] = (dst_local[slot] == iota) on DVE (one op per block)
      PSUM agg = I^T @ hp_b + sum_j S_j^T @ msg_j   (TensorE)
      HS_next[b] = c1 * agg on ACT (c1 = (1-alpha)*norm_src*norm_dst)
      (last iteration: out[b] = c1L * agg, c1L = (1-alpha)*norm_dst)
    then AllGather the new shard into the other table buffer.
"""

import sys

if "/opt/trn_rl_repo" not in sys.path:
    sys.path.insert(0, "/opt/trn_rl_repo")

import numpy as np
import ml_dtypes

import concourse.bass as bass
import concourse.mybir as mybir
import concourse.tile as tile
from concourse import bacc, bass_utils

P = 128
ALPHA = 0.1
# fp16 for the propagation table / messages / S and the MLP matmuls:
# halves gather bytes and PE streaming vs f32, with ~4x better mantissa
# than bf16 (values here are O(1-30), far from fp16 range limits).
PROP_NP = np.float16


class Cfg:
    def __init__(self, n_nodes, in_feats, hidden, n_classes, k_iters, t_q):
        assert n_nodes % 8 == 0
        self.N = n_nodes
        self.FIN = in_feats
        self.HID = hidden
        self.C = n_classes
        self.K = k_iters
        self.NCORES = 8
        self.G = 4                                  # blocks per gather group
        self.R = n_nodes // 8                       # real nodes per core
        nb0 = (self.R + P - 1) // P
        self.NB = ((nb0 + self.G - 1) // self.G) * self.G   # blocks per core
        self.RP = self.NB * P                       # padded rows per core
        self.NCHUNK = 4
        assert self.RP % self.NCHUNK == 0
        self.QR = self.RP // self.NCHUNK            # shard rows per quarter
        self.QB = self.NB // self.NCHUNK            # blocks per quarter
        self.CHUNK_ROWS = 8 * self.QR               # rows per chunk table
        assert self.CHUNK_ROWS <= 32768, "gather chunk must fit int16 indices"
        self.T_q = t_q                              # padded slots per (block, chunk)
        assert t_q % P == 0
        self.CQ = t_q // P                          # msg cols per (block, chunk)
        self.NKC = self.NCHUNK * self.CQ            # msg cols per block
        assert self.NB % self.G == 0
        self.NGRP = self.NB // self.G
        self.IDXC = (self.G * t_q) // 16            # idx cols per gather call
        # MLP row tiling: tiles of 512 rows over ceil(RP/512)*512 padded rows
        self.MLP_PAD = ((self.RP + 511) // 512) * 512
        self.cell_nonempty = [True] * (self.NB * self.NCHUNK)
        assert self.FIN % P == 0 and self.HID % P == 0 and self.C == P


def preprocess(features, edge_index, W1, b1, W2, b2, cfg=None, k_iters=10):
    """Build per-core in_maps + the config. All numpy, not HW-timed."""
    features = np.asarray(features, np.float32)
    ei = np.asarray(edge_index, np.int64)
    W1 = np.asarray(W1, np.float32)
    b1 = np.asarray(b1, np.float32)
    W2 = np.asarray(W2, np.float32)
    b2 = np.asarray(b2, np.float32)
    src, dst = ei[0], ei[1]
    N, FIN = features.shape
    HID = W1.shape[1]
    C = W2.shape[1]

    deg_out = np.bincount(src, minlength=N).astype(np.float64)
    deg_in = np.bincount(dst, minlength=N).astype(np.float64)
    norm_src = (1.0 / np.sqrt(np.maximum(deg_out, 1.0))).astype(np.float32)
    norm_dst = (1.0 / np.sqrt(np.maximum(deg_in, 1.0))).astype(np.float32)

    tmp = Cfg(N, FIN, HID, C, k_iters, P)  # T_q placeholder
    R, RP, NB, NCHUNK = tmp.R, tmp.RP, tmp.NB, tmp.NCHUNK
    QR = tmp.QR

    core_of = dst // R
    # chunk table q holds quarter q of every shard: row = shard*QR + (local - q*QR)
    s_shard = src // R
    s_local = src % R
    chunk = s_local // QR
    src_cidx = s_shard * QR + s_local - chunk * QR  # row within chunk table

    # global T_q: max edges per (core, block, chunk)
    dst_local = dst - core_of * R
    block = dst_local // P
    cell_global = (core_of * NB + block) * NCHUNK + chunk
    counts = np.bincount(cell_global, minlength=8 * NB * NCHUNK)
    t_q = int(((counts.max() + P - 1) // P) * P)
    cfg = Cfg(N, FIN, HID, C, k_iters, t_q)
    # cells empty on EVERY core get no gather call at all (static skip)
    cfg.cell_nonempty = (
        counts.reshape(8, NB * NCHUNK).max(axis=0) > 0
    ).tolist()

    in_maps = []
    for c in range(8):
        m = {}
        sel = core_of == c
        e_srcidx = src_cidx[sel]
        e_chunk = chunk[sel]
        e_block = block[sel]
        e_dstp = dst_local[sel] % P

        cell = e_block * NCHUNK + e_chunk
        # ascending source row within each run -> DMA descriptors hit HBM in
        # address order (page locality for the random gather)
        order = np.lexsort((e_srcidx, cell))
        cell_s = cell[order]
        ccounts = np.bincount(cell_s, minlength=NB * NCHUNK)
        cstarts = np.concatenate([[0], np.cumsum(ccounts)[:-1]])
        within = np.arange(len(cell_s)) - cstarts[cell_s]
        slot = cell_s * t_q + within

        # pads gather row 0 of their chunk (finite data; S excludes them)
        idx_flat = np.zeros(NB * NCHUNK * t_q, np.int16)
        idx_flat[slot] = e_srcidx[order].astype(np.int16)
        dstf_flat = np.full(NB * NCHUNK * t_q, -1.0, np.float32)
        dstf_flat[slot] = e_dstp[order].astype(np.float32)

        # gather idx stream per (group g, chunk q): G consecutive block runs;
        # wrap: slot i -> [i%16, i//16], replicated across the 8 groups of 16
        a = idx_flat.reshape(cfg.NGRP, cfg.G, NCHUNK, t_q)
        a = a.transpose(0, 2, 1, 3).reshape(cfg.NGRP, NCHUNK, cfg.G * t_q)
        w = a.reshape(cfg.NGRP, NCHUNK, (cfg.G * t_q) // 16, 16).transpose(0, 1, 3, 2)
        w = np.broadcast_to(w[:, :, None, :, :], (cfg.NGRP, NCHUNK, 8, 16, cfg.IDXC))
        gidx = np.ascontiguousarray(
            w.reshape(cfg.NGRP, NCHUNK, P, cfg.IDXC).transpose(2, 0, 1, 3)
        ).reshape(P, cfg.NGRP * NCHUNK * cfg.IDXC)
        m["gidx"] = gidx

        # dstf for S build: [128, NB * NKC], col = b*NKC + q*CQ + jq, partition = k
        d = dstf_flat.reshape(NB, NCHUNK, cfg.CQ, P)
        m["dstf"] = np.ascontiguousarray(d.transpose(3, 0, 1, 2)).reshape(
            P, NB * cfg.NKC
        )

        # per-node scales in [128, NB] block-column layout (pad rows -> 0)
        g0 = c * R
        nsrc = np.zeros(RP, np.float32)
        ndst = np.zeros(RP, np.float32)
        nsrc[:R] = norm_src[g0 : g0 + R]
        ndst[:R] = norm_dst[g0 : g0 + R]
        valid = np.zeros(RP, np.float32)
        valid[:R] = 1.0

        def col(x):
            return np.ascontiguousarray(x.reshape(NB, P).T)

        m["c1"] = col((1.0 - ALPHA) * nsrc * ndst)
        m["c1l"] = col((1.0 - ALPHA) * ndst)
        with np.errstate(divide="ignore", invalid="ignore"):
            c3 = np.where(ndst > 0, (ALPHA / (1.0 - ALPHA)) / ndst, 0.0)
        m["c3"] = col(c3.astype(np.float32) * valid)
        m["cs"] = col(nsrc)

        ft = np.zeros((cfg.MLP_PAD, FIN), np.float32)
        ft[:R] = features[g0 : g0 + R]
        m["ft"] = np.ascontiguousarray(ft.T).astype(PROP_NP)
        m["w1"] = W1.astype(PROP_NP)
        m["w2"] = W2.astype(PROP_NP)
        m["b1c"] = np.ascontiguousarray(b1.reshape(HID // P, P).T).astype(np.float32)
        m["b2r"] = b2.reshape(1, C).astype(PROP_NP)
        m["iotaf"] = np.tile(np.arange(P, dtype=np.float32), (P, 1))
        m["identf"] = np.eye(P, dtype=np.float32)
        in_maps.append(m)
    return cfg, in_maps


def build_graph(cfg: Cfg):
    nc = bacc.Bacc("TRN2", target_bir_lowering=False, debug=False)
    f32 = mybir.dt.float32
    f16 = mybir.dt.float16
    i16 = mybir.dt.int16

    ft = nc.declare_dram_parameter("ft", [cfg.FIN, cfg.MLP_PAD], f16, isOutput=False)
    w1 = nc.declare_dram_parameter("w1", [cfg.FIN, cfg.HID], f16, isOutput=False)
    w2 = nc.declare_dram_parameter("w2", [cfg.HID, cfg.C], f16, isOutput=False)
    b1c = nc.declare_dram_parameter("b1c", [P, cfg.HID // P], f32, isOutput=False)
    b2r = nc.declare_dram_parameter("b2r", [1, cfg.C], f16, isOutput=False)
    gidx = nc.declare_dram_parameter(
        "gidx", [P, cfg.NGRP * cfg.NCHUNK * cfg.IDXC], i16, isOutput=False
    )
    dstf = nc.declare_dram_parameter("dstf", [P, cfg.NB * cfg.NKC], f32, isOutput=False)
    c1 = nc.declare_dram_parameter("c1", [P, cfg.NB], f32, isOutput=False)
    c1l = nc.declare_dram_parameter("c1l", [P, cfg.NB], f32, isOutput=False)
    c3 = nc.declare_dram_parameter("c3", [P, cfg.NB], f32, isOutput=False)
    cs = nc.declare_dram_parameter("cs", [P, cfg.NB], f32, isOutput=False)
    iotaf = nc.declare_dram_parameter("iotaf", [P, P], f32, isOutput=False)
    identf = nc.declare_dram_parameter("identf", [P, P], f32, isOutput=False)
    out = nc.declare_dram_parameter("out", [cfg.RP, cfg.C], f32, isOutput=True)

    NKB = cfg.FIN // P   # k-chunks in matmul 1
    NMB = cfg.HID // P   # m-chunks (hidden blocks)
    TROWS = cfg.RP * 8

    with tile.TileContext(nc) as tc:
        with (
            tc.tile_pool(name="const", bufs=1) as cp,
            tc.tile_pool(name="dram", bufs=1, space="DRAM") as dp,
        ):
            # persistent SBUF state
            dstf_t = cp.tile([P, cfg.NB * cfg.NKC], f32)
            c1_t = cp.tile([P, cfg.NB], f32)
            c1l_t = cp.tile([P, cfg.NB], f32)
            c3_t = cp.tile([P, cfg.NB], f32)
            cs_t = cp.tile([P, cfg.NB], f32)
            iota_t = cp.tile([P, P], f32)
            ident_t = cp.tile([P, P], f32)
            ones_t = cp.tile([1, 512], f16)
            b2_t = cp.tile([1, cfg.C], f16)
            b1_t = cp.tile([P, NMB], f32)
            gidx_t = cp.tile([P, cfg.NGRP * cfg.NCHUNK * cfg.IDXC], i16)
            for t, s in [
                (gidx_t, gidx), (dstf_t, dstf), (c1_t, c1), (c1l_t, c1l),
                (c3_t, c3), (cs_t, cs), (iota_t, iotaf), (ident_t, identf),
                (b2_t, b2r), (b1_t, b1c),
            ]:
                nc.sync.dma_start(out=t[:], in_=s[:, :])
            nc.vector.memset(ones_t[:], 1.0)

            # persistent DRAM state: gather tables split into 4 chunk tensors
            # (quarter q of every shard), ping/pong; AllGathers fire per
            # quarter as soon as its 25 blocks are written, overlapping the
            # collective with the remaining compute of the iteration.
            tbl = [
                [
                    nc.dram_tensor(
                        f"tbl{pp}_{q}", [cfg.CHUNK_ROWS, cfg.C], f16,
                        addr_space="Shared",
                    )
                    for q in range(cfg.NCHUNK)
                ]
                for pp in range(2)
            ]
            hs_q = [
                nc.dram_tensor(f"hsq{q}", [cfg.QR, cfg.C], f16)
                for q in range(cfg.NCHUNK)
            ]
            hp_dram = dp.tile([cfg.RP, cfg.C], f32)

            # ---------------- MLP ----------------
            with (
                tc.tile_pool(name="mlp_w", bufs=1) as wp,
                tc.tile_pool(name="mlp_sb", bufs=2) as mp,
                tc.tile_pool(name="mlp_ps", bufs=2, space="PSUM") as pp,
                tc.tile_pool(name="mlp_ps2", bufs=2, space="PSUM") as pp2,
            ):
                w1_t = wp.tile([P, NKB * cfg.HID], f16)
                for k in range(NKB):
                    nc.sync.dma_start(
                        out=w1_t[:, k * cfg.HID : (k + 1) * cfg.HID],
                        in_=w1[k * P : (k + 1) * P, :],
                    )
                w2_t = wp.tile([P, NMB * cfg.C], f16)
                for mth in range(NMB):
                    nc.sync.dma_start(
                        out=w2_t[:, mth * cfg.C : (mth + 1) * cfg.C],
                        in_=w2[mth * P : (mth + 1) * P, :],
                    )

                for t in range(cfg.MLP_PAD // 512):
                    r0 = t * 512
                    ftt = mp.tile([P, NKB * 512], f16, tag="ftt")
                    for k in range(NKB):
                        nc.sync.dma_start(
                            out=ftt[:, k * 512 : (k + 1) * 512],
                            in_=ft[k * P : (k + 1) * P, r0 : r0 + 512],
                        )
                    h_t = mp.tile([P, NMB * 512], f16, tag="h")
                    for mth in range(NMB):
                        ph = pp.tile([P, 512], f32, tag="ph", space="PSUM")
                        for k in range(NKB):
                            nc.tensor.matmul(
                                out=ph[:],
                                lhsT=w1_t[:, k * cfg.HID + mth * P : k * cfg.HID + (mth + 1) * P],
                                rhs=ftt[:, k * 512 : (k + 1) * 512],
                                start=(k == 0),
                                stop=(k == NKB - 1),
                            )
                        nc.scalar.activation(
                            out=h_t[:, mth * 512 : (mth + 1) * 512],
                            in_=ph[:],
                            func=mybir.ActivationFunctionType.Relu,
                            bias=b1_t[:, mth : mth + 1],
                        )
                    ph0 = pp.tile([P, 512], f32, tag="ph0", space="PSUM")
                    nc.tensor.matmul(
                        out=ph0[:], lhsT=b2_t[:], rhs=ones_t[:],
                        start=True, stop=False,
                    )
                    for mth in range(NMB):
                        nc.tensor.matmul(
                            out=ph0[:],
                            lhsT=w2_t[:, mth * cfg.C : (mth + 1) * cfg.C],
                            rhs=h_t[:, mth * 512 : (mth + 1) * 512],
                            start=False,
                            stop=(mth == NMB - 1),
                        )
                    h0T_t = mp.tile([P, 512], f32, tag="h0T")
                    nc.vector.tensor_copy(out=h0T_t[:], in_=ph0[:])
                    for q in range(4):
                        b = t * 4 + q
                        if b >= cfg.NB:
                            break
                        ptr = pp2.tile([P, P], f32, tag="ptr", space="PSUM")
                        nc.tensor.transpose(
                            out=ptr[:],
                            in_=h0T_t[:, q * P : (q + 1) * P],
                            identity=ident_t[:],
                        )
                        hp_sb = mp.tile([P, cfg.C], f32, tag="hp_sb")
                        nc.scalar.activation(
                            out=hp_sb[:], in_=ptr[:],
                            func=mybir.ActivationFunctionType.Copy,
                            scale=c3_t[:, b : b + 1],
                        )
                        nc.sync.dma_start(
                            out=hp_dram[b * P : (b + 1) * P, :], in_=hp_sb[:]
                        )
                        hs_sb = mp.tile([P, cfg.C], f16, tag="hs_sb")
                        nc.scalar.activation(
                            out=hs_sb[:], in_=ptr[:],
                            func=mybir.ActivationFunctionType.Copy,
                            scale=cs_t[:, b : b + 1],
                        )
                        qk, bq = divmod(b, cfg.QB)
                        nc.sync.dma_start(
                            out=hs_q[qk][bq * P : (bq + 1) * P, :], in_=hs_sb[:]
                        )

            rg = [list(range(8))]
            for q in range(cfg.NCHUNK):
                nc.gpsimd.collective_compute(
                    "AllGather", mybir.AluOpType.bypass, replica_groups=rg,
                    ins=[hs_q[q][:, :].opt()], outs=[tbl[0][q][:, :].opt()],
                )

            # ---------------- propagation ----------------
            with (
                tc.tile_pool(name="pr_sb", bufs=2) as sp,
                tc.tile_pool(name="pr_sm", bufs=3) as sp3,
                tc.tile_pool(name="pr_ps", bufs=2, space="PSUM") as ppr,
            ):
                for it in range(cfg.K):
                    cur = tbl[it % 2]
                    nxt = tbl[(it + 1) % 2]
                    last = it == cfg.K - 1
                    for g in range(cfg.NGRP):
                        msg_t = sp.tile([P, cfg.G * cfg.NKC * cfg.C], f16, tag="msg")
                        for q in range(cfg.NCHUNK):
                            icol = (g * cfg.NCHUNK + q) * cfg.IDXC
                            mcol = q * (cfg.G * cfg.CQ) * cfg.C
                            nc.gpsimd.dma_gather(
                                out_ap=msg_t[
                                    :, mcol : mcol + cfg.G * cfg.CQ * cfg.C
                                ].rearrange("p (n c) -> p n c", c=cfg.C),
                                in_ap=cur[q][:, :],
                                idxs_ap=gidx_t[:, icol : icol + cfg.IDXC],
                                num_idxs=cfg.G * cfg.T_q,
                                num_idxs_reg=cfg.G * cfg.T_q,
                                elem_size=cfg.C,
                                single_packet=False,
                            )
                        for blk in range(cfg.G):
                            b = g * cfg.G + blk
                            s_t = sp3.tile([P, cfg.NKC * P], f16, tag="S")
                            dcol = b * cfg.NKC
                            nc.vector.tensor_tensor(
                                out=s_t[:].rearrange("p (n d) -> p n d", n=cfg.NKC),
                                in0=dstf_t[:, dcol : dcol + cfg.NKC]
                                .rearrange("p (n o) -> p n o", o=1)
                                .broadcast_to([P, cfg.NKC, P]),
                                in1=iota_t[:]
                                .rearrange("p (o d) -> p o d", o=1)
                                .broadcast_to([P, cfg.NKC, P]),
                                op=mybir.AluOpType.is_equal,
                            )
                            hp_t = sp3.tile([P, cfg.C], f32, tag="hp")
                            nc.sync.dma_start(
                                out=hp_t[:], in_=hp_dram[b * P : (b + 1) * P, :]
                            )
                            agg = ppr.tile([P, cfg.C], f32, tag="agg", space="PSUM")
                            nc.tensor.matmul(
                                out=agg[:], lhsT=ident_t[:], rhs=hp_t[:],
                                start=True, stop=False,
                            )
                            for j in range(cfg.NKC):
                                q, jq = divmod(j, cfg.CQ)
                                mcol = (q * cfg.G * cfg.CQ + blk * cfg.CQ + jq) * cfg.C
                                nc.tensor.matmul(
                                    out=agg[:],
                                    lhsT=s_t[:, j * P : (j + 1) * P],
                                    rhs=msg_t[:, mcol : mcol + cfg.C],
                                    start=False,
                                    stop=(j == cfg.NKC - 1),
                                )
                            new_t = sp3.tile(
                                [P, cfg.C], f32 if last else f16, tag="new"
                            )
                            nc.scalar.activation(
                                out=new_t[:], in_=agg[:],
                                func=mybir.ActivationFunctionType.Copy,
                                scale=(c1l_t if last else c1_t)[:, b : b + 1],
                            )
                            if last:
                                nc.sync.dma_start(
                                    out=out[b * P : (b + 1) * P, :], in_=new_t[:]
                                )
                            else:
                                qk, bq = divmod(b, cfg.QB)
                                nc.sync.dma_start(
                                    out=hs_q[qk][bq * P : (bq + 1) * P, :],
                                    in_=new_t[:],
                                )
                                if (b + 1) % cfg.QB == 0:
                                    nc.gpsimd.collective_compute(
                                        "AllGather", mybir.AluOpType.bypass,
                                        replica_groups=rg,
                                        ins=[hs_q[qk][:, :].opt()],
                                        outs=[nxt[qk][:, :].opt()],
                                    )

    nc.compile()
    return nc


_CACHE = {}


def _get_compiled(cfg: Cfg):
    key = (cfg.N, cfg.FIN, cfg.HID, cfg.C, cfg.K, cfg.T_q)
    if key not in _CACHE:
        _CACHE[key] = build_graph(cfg)
    return _CACHE[key]


def run(inputs, trace=False):
    cfg, in_maps = preprocess(
        inputs["features"], inputs["edge_index"], inputs["W1"], inputs["b1"],
        inputs["W2"], inputs["b2"],
    )
    nc = _get_compiled(cfg)
    res = bass_utils.run_bass_kernel_spmd(
        nc, in_maps, core_ids=list(range(8)), trace=trace
    )
    parts = [res.results[c]["out"][: cfg.R] for c in range(8)]
    full = np.concatenate(parts, axis=0).astype(np.float32)
    return full, res


def kernel(**inputs) -> np.ndarray:
    full, _ = run(inputs, trace=False)
    return full



# revision 6
# speedup vs baseline: 2.9653x; 2.9653x over previous
"""APPNP (MLP + K-step personalized-pagerank propagation) on 8 TRN2 NeuronCores.

Strategy
--------
Host (numpy, outside HW-timed region):
  - compute degrees / symmetric norms from edge_index
  - shard nodes across 8 cores (12500 each, padded to 12800 = 100 blocks of 128)
  - shard edges by dst core; per core, group edges into gather calls keyed by
    (dst group of 4 blocks, src table-chunk of 25600 rows); within a call the
    4 blocks' edges are PACKED back-to-back (real per-core counts, sorted by
    src for HBM locality) and padded with trailing -1 up to a shared static
    T_call (max over cores).  The gather ucode trims trailing negative
    indices, so descriptor generation only pays for real edges.
  - per (group, block) build the dstf stream (values block_in_group*128 +
    dst%128, -1 on pads) in the exact tile order the device consumes, so one
    DVE is_equal op per block builds all its selection-matrix tiles.

Device (one Bass graph, SPMD on 8 cores):
  - MLP in fp16 on TensorE: h0^T tiles = W2^T @ relu(W1^T @ X^T + b1) + b2
  - per 128-node block: transpose h0^T -> node-major, keep
      hp   = (alpha/(1-alpha)) * h0 / norm_dst     (SBUF-resident, all iters)
      HS_0 = norm_src * h0                         (shard of gather table)
  - AllGather shard quarters -> full gather table (ping/pong DRAM chunks)
  - K=10 propagation iterations, per group g of 4 blocks:
      4 packed dma_gather calls, chunk q on SWDGE queue q -> the 4 Q7 core
      pairs generate descriptors in parallel (this is the critical path)
      per block: one DVE is_equal op builds S over the block's tile range,
      PSUM agg = I^T @ hp_b + sum_tiles S_t^T @ msg_t   (TensorE)
      new = c1 * agg on ACT; DMA to the AllGather quarter (or out on the
      last iteration); AllGather fires per completed quarter.
"""

import os
import sys

if "/opt/trn_rl_repo" not in sys.path:
    sys.path.insert(0, "/opt/trn_rl_repo")

# test-only override: force all gathers onto one SWDGE queue (sim support)
_NQ = int(os.environ.get("KERNEL_NQ", "4"))

import numpy as np

import concourse.bass as bass
import concourse.mybir as mybir
import concourse.tile as tile
from concourse import bacc, bass_utils

P = 128
ALPHA = 0.1
PROP_NP = np.float16


class Cfg:
    def __init__(self, n_nodes, in_feats, hidden, n_classes, k_iters):
        assert n_nodes % 8 == 0
        self.N = n_nodes
        self.FIN = in_feats
        self.HID = hidden
        self.C = n_classes
        self.K = k_iters
        self.NCORES = 8
        self.G = 4                                  # blocks per gather group
        self.R = n_nodes // 8                       # real nodes per core
        nb0 = (self.R + P - 1) // P
        self.NB = ((nb0 + self.G - 1) // self.G) * self.G   # blocks per core
        self.RP = self.NB * P                       # padded rows per core
        self.NCHUNK = 4
        assert self.RP % self.NCHUNK == 0
        self.QR = self.RP // self.NCHUNK            # shard rows per quarter
        self.QB = self.NB // self.NCHUNK            # blocks per quarter
        self.CHUNK_ROWS = 8 * self.QR               # rows per chunk table
        assert self.CHUNK_ROWS <= 32768, "gather chunk must fit int16 indices"
        assert self.NB % self.G == 0
        self.NGRP = self.NB // self.G
        self.MLP_PAD = ((self.RP + 511) // 512) * 512
        assert self.FIN % P == 0 and self.HID % P == 0 and self.C == P
        # filled by preprocess:
        self.T_call = None      # [NGRP][NCHUNK] padded idx count per call
        self.tile_rng = None    # [NGRP][NCHUNK][G] -> (lo, hi) or None
        self.Wsum = None        # [NGRP][G] total S tiles for block
        self.key_extra = None

    def key(self):
        return (self.N, self.FIN, self.HID, self.C, self.K, self.key_extra)


def preprocess(features, edge_index, W1, b1, W2, b2, k_iters=10):
    features = np.asarray(features, np.float32)
    ei = np.asarray(edge_index, np.int64)
    W1 = np.asarray(W1, np.float32)
    b1 = np.asarray(b1, np.float32)
    W2 = np.asarray(W2, np.float32)
    b2 = np.asarray(b2, np.float32)
    src, dst = ei[0], ei[1]
    N, FIN = features.shape
    HID = W1.shape[1]
    C = W2.shape[1]

    deg_out = np.bincount(src, minlength=N).astype(np.float64)
    deg_in = np.bincount(dst, minlength=N).astype(np.float64)
    norm_src = (1.0 / np.sqrt(np.maximum(deg_out, 1.0))).astype(np.float32)
    norm_dst = (1.0 / np.sqrt(np.maximum(deg_in, 1.0))).astype(np.float32)

    cfg = Cfg(N, FIN, HID, C, k_iters)
    R, NB, G, NCHUNK, NGRP, QR = cfg.R, cfg.NB, cfg.G, cfg.NCHUNK, cfg.NGRP, cfg.QR

    core_of = dst // R
    s_shard = src // R
    s_local = src % R
    chunk = s_local // QR
    src_cidx = s_shard * QR + s_local - chunk * QR   # row within chunk table

    dst_local = dst - core_of * R
    block = dst_local // P
    grp = block // G
    bing = block % G
    dst_p = dst_local % P

    # per-core per-call packed edge lists -------------------------------
    # sort edges by (core, grp, chunk, bing, src_cidx)
    order = np.lexsort((src_cidx, bing, chunk, grp, core_of))
    so_core = core_of[order]
    so_grp = grp[order]
    so_chunk = chunk[order]
    so_bing = bing[order]
    so_cidx = src_cidx[order]
    so_dstp = dst_p[order]

    # counts[core, grp, chunk, bing]
    cell_id = ((so_core * NGRP + so_grp) * NCHUNK + so_chunk) * G + so_bing
    counts = np.bincount(cell_id, minlength=8 * NGRP * NCHUNK * G).reshape(
        8, NGRP, NCHUNK, G
    )
    call_cnt = counts.sum(axis=3)                  # [8, NGRP, NCHUNK]
    T_call = (
        np.ceil(np.maximum(call_cnt.max(axis=0), 1) / P).astype(np.int64) * P
    )                                              # [NGRP, NCHUNK]

    # per-core block offsets within each call
    cum = np.cumsum(counts, axis=3)
    off = cum - counts                             # start of block within call

    # union tile ranges over cores (only cores with cnt>0 contribute)
    lo_t = np.where(counts > 0, off // P, np.iinfo(np.int64).max)
    hi_t = np.where(counts > 0, (cum - 1) // P, -1)
    lo_u = lo_t.min(axis=0)                        # [NGRP, NCHUNK, G]
    hi_u = hi_t.max(axis=0)

    tile_rng = [
        [
            [
                (int(lo_u[g, q, j]), int(hi_u[g, q, j]))
                if hi_u[g, q, j] >= 0
                else None
                for j in range(G)
            ]
            for q in range(NCHUNK)
        ]
        for g in range(NGRP)
    ]
    Wsum = [
        [
            sum(
                (tile_rng[g][q][j][1] - tile_rng[g][q][j][0] + 1)
                if tile_rng[g][q][j]
                else 0
                for q in range(NCHUNK)
            )
            for j in range(G)
        ]
        for g in range(NGRP)
    ]
    cfg.T_call = T_call.tolist()
    cfg.tile_rng = tile_rng
    cfg.Wsum = Wsum
    cfg.IDX_COLS = int(T_call.sum() // 16)         # gidx cols per core
    cfg.DSTF_COLS = int(sum(sum(w for w in Wsum[g]) for g in range(NGRP)))
    cfg.WMAX = max(max(w for w in Wsum[g]) for g in range(NGRP))
    cfg.TMAXQ = [int(T_call[:, q].max()) for q in range(NCHUNK)]
    cfg.key_extra = (tuple(T_call.ravel().tolist()), tuple(lo_u.ravel().tolist()),
                     tuple(hi_u.ravel().tolist()))

    # call/col offsets (static across cores)
    idx_off = np.zeros((NGRP, NCHUNK), np.int64)   # in int16 cols (16-wrapped)
    acc = 0
    for g in range(NGRP):
        for q in range(NCHUNK):
            idx_off[g, q] = acc
            acc += T_call[g, q] // 16
    dstf_off = np.zeros((NGRP, G), np.int64)       # in tile cols
    acc = 0
    for g in range(NGRP):
        for j in range(G):
            dstf_off[g, j] = acc
            acc += Wsum[g][j]
    cfg.idx_off = idx_off.tolist()
    cfg.dstf_off = dstf_off.tolist()

    # per-core input tensors -------------------------------------------
    core_starts = np.searchsorted(so_core, np.arange(9))
    in_maps = []
    for c in range(8):
        m = {}
        lo_e, hi_e = core_starts[c], core_starts[c + 1]
        e_grp = so_grp[lo_e:hi_e]
        e_chunk = so_chunk[lo_e:hi_e]
        e_bing = so_bing[lo_e:hi_e]
        e_cidx = so_cidx[lo_e:hi_e]
        e_dstp = so_dstp[lo_e:hi_e]

        gidx = np.zeros((P, cfg.IDX_COLS), np.int16)
        dstf = np.full((P, cfg.DSTF_COLS), -1.0, np.float16)

        # edges are already sorted by (grp, chunk, bing, src)
        call_of_e = e_grp * NCHUNK + e_chunk
        call_starts = np.searchsorted(call_of_e, np.arange(NGRP * NCHUNK + 1))
        for g in range(NGRP):
            for q in range(NCHUNK):
                cid = g * NCHUNK + q
                a, b = call_starts[cid], call_starts[cid + 1]
                T = cfg.T_call[g][q]
                idx = np.zeros(T, np.int16)   # pads gather row 0 (finite)
                idx[: b - a] = e_cidx[a:b].astype(np.int16)
                # wrap: slot i -> [i%16, i//16], replicated across 8 groups
                w = idx.reshape(T // 16, 16).T
                col0 = cfg.idx_off[g][q]
                gidx[:, col0 : col0 + T // 16] = np.broadcast_to(
                    w[None], (8, 16, T // 16)
                ).reshape(P, T // 16)
                # dstf values in call-slot order
                dv = np.full(T, -1.0, np.float16)
                dv[: b - a] = (e_bing[a:b] * P + e_dstp[a:b]).astype(np.float16)
                dvt = dv.reshape(T // P, P).T       # [P, tiles] partition=slot%128
                for j in range(G):
                    rng = cfg.tile_rng[g][q][j]
                    if rng is None:
                        continue
                    lo_j, hi_j = rng
                    pos = cfg.dstf_off[g][j] + sum(
                        (cfg.tile_rng[g][qq][j][1] - cfg.tile_rng[g][qq][j][0] + 1)
                        if (qq < q and cfg.tile_rng[g][qq][j])
                        else 0
                        for qq in range(NCHUNK)
                    )
                    dstf[:, pos : pos + hi_j - lo_j + 1] = dvt[:, lo_j : hi_j + 1]
        m["gidx"] = gidx
        m["dstf"] = dstf

        # per-node scales in [128, NB] block-column layout (pad rows -> 0)
        g0 = c * R
        nsrc = np.zeros(cfg.RP, np.float32)
        ndst = np.zeros(cfg.RP, np.float32)
        nsrc[:R] = norm_src[g0 : g0 + R]
        ndst[:R] = norm_dst[g0 : g0 + R]
        valid = np.zeros(cfg.RP, np.float32)
        valid[:R] = 1.0

        def col(x):
            return np.ascontiguousarray(x.reshape(NB, P).T)

        m["c1"] = col((1.0 - ALPHA) * nsrc * ndst)
        m["c1l"] = col((1.0 - ALPHA) * ndst)
        with np.errstate(divide="ignore", invalid="ignore"):
            c3 = np.where(ndst > 0, (ALPHA / (1.0 - ALPHA)) / ndst, 0.0)
        m["c3"] = col(c3.astype(np.float32) * valid)
        m["cs"] = col(nsrc)

        ft = np.zeros((cfg.MLP_PAD, FIN), np.float32)
        ft[:R] = features[g0 : g0 + R]
        m["ft"] = np.ascontiguousarray(ft.T).astype(PROP_NP)
        m["w1"] = W1.astype(PROP_NP)
        m["w2"] = W2.astype(PROP_NP)
        m["b1c"] = np.ascontiguousarray(b1.reshape(HID // P, P).T).astype(np.float32)
        m["b2r"] = b2.reshape(1, C).astype(PROP_NP)
        # iota rows: iotag[p, j*P + d] = j*128 + d  (f16-exact up to 511)
        iarow = (
            np.arange(cfg.G, dtype=np.float32)[:, None] * P
            + np.arange(P, dtype=np.float32)[None, :]
        ).reshape(1, cfg.G * P)
        m["iotag"] = np.broadcast_to(iarow, (P, cfg.G * P)).astype(np.float16)
        m["identf"] = np.eye(P, dtype=np.float32)
        in_maps.append(m)
    return cfg, in_maps


def build_graph(cfg: Cfg):
    nc = bacc.Bacc(
        "TRN2", target_bir_lowering=False, debug=False, num_swdge_queues=4
    )
    f32 = mybir.dt.float32
    f16 = mybir.dt.float16
    i16 = mybir.dt.int16

    ft = nc.declare_dram_parameter("ft", [cfg.FIN, cfg.MLP_PAD], f16, isOutput=False)
    w1 = nc.declare_dram_parameter("w1", [cfg.FIN, cfg.HID], f16, isOutput=False)
    w2 = nc.declare_dram_parameter("w2", [cfg.HID, cfg.C], f16, isOutput=False)
    b1c = nc.declare_dram_parameter("b1c", [P, cfg.HID // P], f32, isOutput=False)
    b2r = nc.declare_dram_parameter("b2r", [1, cfg.C], f16, isOutput=False)
    gidx = nc.declare_dram_parameter("gidx", [P, cfg.IDX_COLS], i16, isOutput=False)
    dstf = nc.declare_dram_parameter("dstf", [P, cfg.DSTF_COLS], f16, isOutput=False)
    c1 = nc.declare_dram_parameter("c1", [P, cfg.NB], f32, isOutput=False)
    c1l = nc.declare_dram_parameter("c1l", [P, cfg.NB], f32, isOutput=False)
    c3 = nc.declare_dram_parameter("c3", [P, cfg.NB], f32, isOutput=False)
    cs = nc.declare_dram_parameter("cs", [P, cfg.NB], f32, isOutput=False)
    iotag = nc.declare_dram_parameter("iotag", [P, cfg.G * P], f16, isOutput=False)
    identf = nc.declare_dram_parameter("identf", [P, P], f32, isOutput=False)
    out = nc.declare_dram_parameter("out", [cfg.RP, cfg.C], f32, isOutput=True)

    NKB = cfg.FIN // P   # k-chunks in matmul 1
    NMB = cfg.HID // P   # m-chunks (hidden blocks)

    with tile.TileContext(nc) as tc:
        with tc.tile_pool(name="const", bufs=1) as cp:
            # persistent SBUF state
            dstf_t = cp.tile([P, cfg.DSTF_COLS], f16)
            c1_t = cp.tile([P, cfg.NB], f32)
            c1l_t = cp.tile([P, cfg.NB], f32)
            c3_t = cp.tile([P, cfg.NB], f32)
            cs_t = cp.tile([P, cfg.NB], f32)
            iotag_t = cp.tile([P, cfg.G * P], f16)
            ident_t = cp.tile([P, P], f32)
            ones_t = cp.tile([1, 512], f16)
            b2_t = cp.tile([1, cfg.C], f16)
            b1_t = cp.tile([P, NMB], f32)
            gidx_t = cp.tile([P, cfg.IDX_COLS], i16)
            hp_t = cp.tile([P, cfg.NB * cfg.C], f32)   # resident alpha-anchor
            for t, s in [
                (gidx_t, gidx), (dstf_t, dstf), (c1_t, c1), (c1l_t, c1l),
                (c3_t, c3), (cs_t, cs), (iotag_t, iotag), (ident_t, identf),
                (b2_t, b2r), (b1_t, b1c),
            ]:
                nc.sync.dma_start(out=t[:], in_=s[:, :])
            nc.vector.memset(ones_t[:], 1.0)

            # persistent DRAM: gather tables split into 4 chunk tensors
            # (quarter q of every shard), ping/pong
            tbl = [
                [
                    nc.dram_tensor(
                        f"tbl{pp}_{q}", [cfg.CHUNK_ROWS, cfg.C], f16,
                        addr_space="Shared",
                    )
                    for q in range(cfg.NCHUNK)
                ]
                for pp in range(2)
            ]
            hs_q = [
                nc.dram_tensor(f"hsq{q}", [cfg.QR, cfg.C], f16)
                for q in range(cfg.NCHUNK)
            ]

            rg = [list(range(8))]

            # ---------------- MLP ----------------
            with (
                tc.tile_pool(name="mlp_w", bufs=1) as wp,
                tc.tile_pool(name="mlp_sb", bufs=2) as mp,
                tc.tile_pool(name="mlp_ps", bufs=2, space="PSUM") as pp,
                tc.tile_pool(name="mlp_ps2", bufs=2, space="PSUM") as pp2,
            ):
                w1_t = wp.tile([P, NKB * cfg.HID], f16)
                for k in range(NKB):
                    nc.sync.dma_start(
                        out=w1_t[:, k * cfg.HID : (k + 1) * cfg.HID],
                        in_=w1[k * P : (k + 1) * P, :],
                    )
                w2_t = wp.tile([P, NMB * cfg.C], f16)
                for mth in range(NMB):
                    nc.sync.dma_start(
                        out=w2_t[:, mth * cfg.C : (mth + 1) * cfg.C],
                        in_=w2[mth * P : (mth + 1) * P, :],
                    )

                for t in range(cfg.MLP_PAD // 512):
                    r0 = t * 512
                    ftt = mp.tile([P, NKB * 512], f16, tag="ftt")
                    for k in range(NKB):
                        nc.sync.dma_start(
                            out=ftt[:, k * 512 : (k + 1) * 512],
                            in_=ft[k * P : (k + 1) * P, r0 : r0 + 512],
                        )
                    h_t = mp.tile([P, NMB * 512], f16, tag="h")
                    for mth in range(NMB):
                        ph = pp.tile([P, 512], f32, tag="ph", space="PSUM")
                        for k in range(NKB):
                            nc.tensor.matmul(
                                out=ph[:],
                                lhsT=w1_t[:, k * cfg.HID + mth * P : k * cfg.HID + (mth + 1) * P],
                                rhs=ftt[:, k * 512 : (k + 1) * 512],
                                start=(k == 0),
                                stop=(k == NKB - 1),
                            )
                        nc.scalar.activation(
                            out=h_t[:, mth * 512 : (mth + 1) * 512],
                            in_=ph[:],
                            func=mybir.ActivationFunctionType.Relu,
                            bias=b1_t[:, mth : mth + 1],
                        )
                    ph0 = pp.tile([P, 512], f32, tag="ph0", space="PSUM")
                    nc.tensor.matmul(
                        out=ph0[:], lhsT=b2_t[:], rhs=ones_t[:],
                        start=True, stop=False,
                    )
                    for mth in range(NMB):
                        nc.tensor.matmul(
                            out=ph0[:],
                            lhsT=w2_t[:, mth * cfg.C : (mth + 1) * cfg.C],
                            rhs=h_t[:, mth * 512 : (mth + 1) * 512],
                            start=False,
                            stop=(mth == NMB - 1),
                        )
                    h0T_t = mp.tile([P, 512], f32, tag="h0T")
                    nc.vector.tensor_copy(out=h0T_t[:], in_=ph0[:])
                    for qq in range(4):
                        b = t * 4 + qq
                        if b >= cfg.NB:
                            break
                        ptr = pp2.tile([P, P], f32, tag="ptr", space="PSUM")
                        nc.tensor.transpose(
                            out=ptr[:],
                            in_=h0T_t[:, qq * P : (qq + 1) * P],
                            identity=ident_t[:],
                        )
                        nc.scalar.activation(
                            out=hp_t[:, b * cfg.C : (b + 1) * cfg.C],
                            in_=ptr[:],
                            func=mybir.ActivationFunctionType.Copy,
                            scale=c3_t[:, b : b + 1],
                        )
                        hs_sb = mp.tile([P, cfg.C], f16, tag="hs_sb")
                        nc.scalar.activation(
                            out=hs_sb[:], in_=ptr[:],
                            func=mybir.ActivationFunctionType.Copy,
                            scale=cs_t[:, b : b + 1],
                        )
                        qk, bq = divmod(b, cfg.QB)
                        nc.sync.dma_start(
                            out=hs_q[qk][bq * P : (bq + 1) * P, :], in_=hs_sb[:]
                        )
                        if bq == cfg.QB - 1:
                            nc.gpsimd.collective_compute(
                                "AllGather", mybir.AluOpType.bypass,
                                replica_groups=rg,
                                ins=[hs_q[qk][:, :].opt()],
                                outs=[tbl[0][qk][:, :].opt()],
                            )

            # ---------------- propagation ----------------
            with (
                tc.tile_pool(name="pr_m0", bufs=2) as mp0,
                tc.tile_pool(name="pr_m1", bufs=2) as mp1,
                tc.tile_pool(name="pr_m2", bufs=2) as mp2,
                tc.tile_pool(name="pr_m3", bufs=2) as mp3,
                tc.tile_pool(name="pr_sm", bufs=3) as sp3,
                tc.tile_pool(name="pr_ps", bufs=4, space="PSUM") as ppr,
            ):
                mpool = [mp0, mp1, mp2, mp3]
                # init msg buffers to finite values (trailing-trimmed gather
                # slots keep stale SBUF contents; S masks them but NaN*0=NaN)
                msg_init = []
                for q in range(cfg.NCHUNK):
                    for _ in range(2):
                        mt = mpool[q].tile(
                            [P, (cfg.TMAXQ[q] // P) * cfg.C], f16, tag=f"msg{q}"
                        )
                        nc.vector.memset(mt[:], 0.0)
                        msg_init.append(mt)

                for it in range(cfg.K):
                    cur = tbl[it % 2]
                    nxt = tbl[(it + 1) % 2]
                    last = it == cfg.K - 1
                    for g in range(cfg.NGRP):
                        msgs = []
                        for q in range(cfg.NCHUNK):
                            T = cfg.T_call[g][q]
                            mt = mpool[q].tile(
                                [P, (cfg.TMAXQ[q] // P) * cfg.C], f16,
                                tag=f"msg{q}",
                            )
                            icol = cfg.idx_off[g][q]
                            nc.gpsimd.dma_gather(
                                out_ap=mt[:, : (T // P) * cfg.C].rearrange(
                                    "p (n c) -> p n c", c=cfg.C
                                ),
                                in_ap=cur[q][:, :],
                                idxs_ap=gidx_t[:, icol : icol + T // 16],
                                num_idxs=T,
                                num_idxs_reg=T,
                                elem_size=cfg.C,
                                single_packet=False,
                                queue_num=q % _NQ,
                            )
                            msgs.append(mt)
                        for j in range(cfg.G):
                            b = g * cfg.G + j
                            W = cfg.Wsum[g][j]
                            if W > 0:
                                s_t = sp3.tile([P, cfg.WMAX * P], f16, tag="S")
                                dcol = cfg.dstf_off[g][j]
                                nc.vector.tensor_tensor(
                                    out=s_t[:, : W * P].rearrange(
                                        "p (n d) -> p n d", n=W
                                    ),
                                    in0=dstf_t[:, dcol : dcol + W]
                                    .rearrange("p (n o) -> p n o", o=1)
                                    .broadcast_to([P, W, P]),
                                    in1=iotag_t[:, j * P : (j + 1) * P]
                                    .rearrange("p (o d) -> p o d", o=1)
                                    .broadcast_to([P, W, P]),
                                    op=mybir.AluOpType.is_equal,
                                )
                            agg = ppr.tile([P, cfg.C], f32, tag="agg", space="PSUM")
                            nc.tensor.matmul(
                                out=agg[:], lhsT=ident_t[:],
                                rhs=hp_t[:, b * cfg.C : (b + 1) * cfg.C],
                                start=True, stop=(W == 0),
                            )
                            ti = 0
                            for q in range(cfg.NCHUNK):
                                rng = cfg.tile_rng[g][q][j]
                                if rng is None:
                                    continue
                                lo_j, hi_j = rng
                                for t in range(lo_j, hi_j + 1):
                                    nc.tensor.matmul(
                                        out=agg[:],
                                        lhsT=s_t[:, ti * P : (ti + 1) * P],
                                        rhs=msgs[q][:, t * cfg.C : (t + 1) * cfg.C],
                                        start=False,
                                        stop=(ti == W - 1),
                                    )
                                    ti += 1
                            new_t = sp3.tile(
                                [P, cfg.C], f32 if last else f16, tag="new"
                            )
                            nc.scalar.activation(
                                out=new_t[:], in_=agg[:],
                                func=mybir.ActivationFunctionType.Copy,
                                scale=(c1l_t if last else c1_t)[:, b : b + 1],
                            )
                            if last:
                                nc.sync.dma_start(
                                    out=out[b * P : (b + 1) * P, :], in_=new_t[:]
                                )
                            else:
                                qk, bq = divmod(b, cfg.QB)
                                nc.sync.dma_start(
                                    out=hs_q[qk][bq * P : (bq + 1) * P, :],
                                    in_=new_t[:],
                                )
                                if bq == cfg.QB - 1:
                                    nc.gpsimd.collective_compute(
                                        "AllGather", mybir.AluOpType.bypass,
                                        replica_groups=rg,
                                        ins=[hs_q[qk][:, :].opt()],
                                        outs=[nxt[qk][:, :].opt()],
                                    )

    nc.compile()
    return nc


_CACHE = {}


def _get_compiled(cfg: Cfg):
    key = cfg.key()
    if key not in _CACHE:
        _CACHE[key] = build_graph(cfg)
    return _CACHE[key]


def run(inputs, trace=False):
    cfg, in_maps = preprocess(
        inputs["features"], inputs["edge_index"], inputs["W1"], inputs["b1"],
        inputs["W2"], inputs["b2"],
    )
    nc = _get_compiled(cfg)
    res = bass_utils.run_bass_kernel_spmd(
        nc, in_maps, core_ids=list(range(8)), trace=trace
    )
    parts = [res.results[c]["out"][: cfg.R] for c in range(8)]
    full = np.concatenate(parts, axis=0).astype(np.float32)
    return full, res


def kernel(**inputs) -> np.ndarray:
    full, _ = run(inputs, trace=False)
    return full
